# revision 6
# baseline (speedup 1.0000x reference)
"""TRN2 Bass kernel for nn_BalancedHamiltonLayer.

Math: out[n,k,j] = sum_{r,s,i} x[n,s,i] * factors_B[r,j,i] * H(A)[r,k,s] + bias
collapses to a single dense matmul  out = x2d @ W + bias  with
W[(s,i),(k,j)] = sum_r H[r,k,s] * B[r,j,i]  (1024x1024, folded on host in f64).

Sharding: data-parallel over the 8192 token rows across 8 NeuronCores
(1024 rows each); W replicated.

Mixed precision: contraction k-chunks {0,2,4,6} (of 8x128) run as fp8e4
DoubleRow matmuls (4x PE rate measured on HW: ~113ns per out[128,256]
K=256 instr): the stationary carries the x chunk duplicated in both
DoubleRow slots, the moving carries a (hi, lo) e4m3 pair of W/1.3 so W
quantization error cancels and only the single-fp8 x error (* sqrt(1/2))
remains.  x for those chunks is quantized as e4m3(1.3*x).  Chunks
{1,3,5,7} stay fp16.  Realized max rel err (exact, fixed seed): 1.69e-2
vs the 2e-2 gate.  PE floor: 8 tiles x (4*1024 + 4*512) cyc = 49152 cyc
= 20.5us @2.4GHz.

Schedule (per core):
- W streams on the scalar HWDGE queue in the phase-1 consumption order
  (alternating fp16/fp8 chunks); x streams on sync.  First pieces are
  small so the first real matmul can start ~9.5us into the window.
- warmup matmuls on a zeroed tile bridge body-start -> first data and
  ramp the PE HAM clock; tunable counts.
- phase 1 interleaves m0..m2 across chunks (3 MMs per W chunk piece);
  phase 2 runs m3..m7 k-contiguous per half with stores draining on the
  scalar (h0) / sync (h1) queues.  The final half is split into a
  quarter + two eighth accumulation groups to shrink the tail chain.
"""

import numpy as np
import ml_dtypes
import concourse.bacc as bacc
import concourse.mybir as mybir
import concourse.tile as tile
from concourse.bass_utils import run_bass_kernel_spmd

F8 = ml_dtypes.float8_e4m3

B, T, D = 4, 2048, 1024
RANK, FACTOR, SUB = 8, 64, 4
S = 4 * SUB  # 16
NCORES = 8
NTOK = B * T // NCORES  # 1024 token rows per core
P = 128
MT = NTOK // P  # 8 token tiles per core
NH = 512        # half of the 1024 output cols (one PSUM bank)

CH8 = (0, 2, 4, 6)   # fp8 k-chunks
CH16 = (1, 3, 5, 7)  # fp16 k-chunks
CX = 1.3             # x fp8 pre-scale (1/CX folded into W before hi/lo)
NC8 = len(CH8)
NC16 = len(CH16)

NWARM_BIG = 4
NWARM_SMALL = 3

_cached_nc = None


def build_module():
    global _cached_nc
    if _cached_nc is not None:
        return _cached_nc
    nc = bacc.Bacc("TRN2", target_bir_lowering=False, debug=False)
    xH16 = nc.dram_tensor("xH16", [4, P, NC16, P], mybir.dt.float16, kind="ExternalInput").ap()
    xH8 = nc.dram_tensor("xH8", [4, P, NC8, 2, P], mybir.dt.float8e4, kind="ExternalInput").ap()
    xB16 = nc.dram_tensor("xB16", [P, 4, NC16, P], mybir.dt.float16, kind="ExternalInput").ap()
    xB8 = nc.dram_tensor("xB8", [P, 4, NC8, 2, P], mybir.dt.float8e4, kind="ExternalInput").ap()
    wH16 = nc.dram_tensor("wH16", [NC16, P, D], mybir.dt.float16, kind="ExternalInput").ap()
    wH8 = nc.dram_tensor("wH8", [NC8, P, 2, D], mybir.dt.float8e4, kind="ExternalInput").ap()
    out = nc.dram_tensor("out", [NTOK, D], mybir.dt.float16, kind="ExternalOutput").ap()

    DR = mybir.MatmulPerfMode.DoubleRow

    with tile.TileContext(nc) as tc:
        with (
            tc.tile_pool(name="wp", bufs=1) as wp,
            tc.tile_pool(name="xp", bufs=1) as xp,
            tc.tile_pool(name="op", bufs=1) as op,
            tc.tile_pool(name="ps", bufs=8, space="PSUM") as ps,
        ):
            g = xp.tile([P, NH], mybir.dt.float16, tag="warm", name="g")
            nc.vector.memset(g[:], 0.0)

            w16t = [wp.tile([P, D], mybir.dt.float16, tag=f"w16_{j}", name=f"w16_{j}")
                    for j in range(NC16)]
            w8t = [wp.tile([P, 2, D], mybir.dt.float8e4, tag=f"w8_{j}", name=f"w8_{j}")
                   for j in range(NC8)]
            x16t = [xp.tile([P, NC16, P], mybir.dt.float16, tag=f"x16_{m}", name=f"x16_{m}")
                    for m in range(4)]
            x8t = [xp.tile([P, NC8, 2, P], mybir.dt.float8e4, tag=f"x8_{m}", name=f"x8_{m}")
                   for m in range(4)]
            xb16 = xp.tile([P, 4, NC16, P], mybir.dt.float16, tag="xb16", name="xb16")
            xb8 = xp.tile([P, 4, NC8, 2, P], mybir.dt.float8e4, tag="xb8", name="xb8")

            # W on the scalar HWDGE queue, in phase-1 consumption order
            # (fp16 chunk j alternating with fp8 chunk j); x on sync.
            nc.scalar.dma_start(w16t[0][:, :NH], wH16[0, :, :NH])
            nc.sync.dma_start(x16t[0][:, 0:1], xH16[0, :, 0:1])
            nc.scalar.dma_start(w16t[0][:, NH:], wH16[0, :, NH:])
            nc.sync.dma_start(x16t[1][:, 0:1], xH16[1, :, 0:1])
            nc.sync.dma_start(x16t[2][:, 0:1], xH16[2, :, 0:1])
            nc.scalar.dma_start(w8t[0][:], wH8[0])
            nc.sync.dma_start(x16t[0][:, 1:], xH16[0, :, 1:])
            nc.sync.dma_start(x8t[0][:], xH8[0])
            nc.scalar.dma_start(w16t[1][:], wH16[1])
            nc.sync.dma_start(x16t[1][:, 1:], xH16[1, :, 1:])
            nc.scalar.dma_start(w8t[1][:], wH8[1])
            nc.sync.dma_start(x8t[1][:], xH8[1])
            nc.scalar.dma_start(w16t[2][:], wH16[2])
            nc.sync.dma_start(x16t[2][:, 1:], xH16[2, :, 1:])
            nc.scalar.dma_start(w8t[2][:], wH8[2])
            nc.sync.dma_start(x8t[2][:], xH8[2])
            nc.scalar.dma_start(w16t[3][:], wH16[3])
            nc.sync.dma_start(x16t[3][:], xH16[3])
            nc.scalar.dma_start(w8t[3][:], wH8[3])
            nc.sync.dma_start(x8t[3][:], xH8[3])
            nc.sync.dma_start(xb16[:], xB16)
            nc.sync.dma_start(xb8[:], xB8)

            def xs16(m, j):
                return x16t[m][:, j, :] if m < 4 else xb16[:, m - 4, j, :]

            def xs8(m, j):
                return x8t[m][:, j] if m < 4 else xb8[:, m - 4, j]

            ot = {}

            def emit_piece(m, c0, c1, pt_ap, eng):
                if m not in ot:
                    ot[m] = op.tile([P, D], mybir.dt.float16, tag=f"o{m}", name=f"o{m}")
                o = ot[m]
                nc.vector.tensor_copy(o[:, c0:c1], pt_ap)
                eng.dma_start(out[m * P:(m + 1) * P, c0:c1], o[:, c0:c1])

            def emit_half(m, h, pt):
                emit_piece(m, h * NH, (h + 1) * NH, pt[:], nc.scalar if h == 0 else nc.sync)

            with nc.named_scope("mm"):
                pts = {
                    m: {h: ps.tile([P, NH], mybir.dt.float32, tag="ps", name=f"pt{m}_{h}")
                        for h in range(2)}
                    for m in range(3)
                }
                for i in range(NWARM_BIG):
                    nc.tensor.matmul(pts[0][0][:], g[:, :P], g[:], start=(i == 0), stop=False)
                for i in range(NWARM_SMALL):
                    nc.tensor.matmul(pts[0][0][:, :P], g[:, :P], g[:, :P], start=False, stop=False)

                def mm16(pt, m, j, h, start=False, stop=False):
                    nc.tensor.matmul(
                        pt[:],
                        xs16(m, j),
                        w16t[j][:, h * NH:(h + 1) * NH],
                        start=start, stop=stop,
                    )

                def mm8(pt, m, j, h, q, stop=False, qw=256):
                    c0 = h * NH + q * qw
                    nc.tensor.matmul(
                        pt[:, q * qw:q * qw + qw],
                        xs8(m, j),
                        w8t[j][:, :, c0:c0 + qw],
                        start=False, stop=stop,
                        perf_mode=DR,
                    )

                # phase 1: m0..m2 interleaved, chunk order = W arrival order.
                # fp16 j then fp8 j, alternating.  m0h0 continues the warmup
                # accumulation group (start stays False).
                for m in (0, 1, 2):
                    nc.tensor.matmul(
                        pts[m][0][:], xs16(m, 0), w16t[0][:, :NH],
                        start=(m != 0), stop=False,
                    )
                for m in (0, 1, 2):
                    nc.tensor.matmul(
                        pts[m][1][:], xs16(m, 0), w16t[0][:, NH:],
                        start=True, stop=False,
                    )
                for j in range(NC8):
                    if j > 0:
                        for m in (0, 1, 2):
                            for h in (0, 1):
                                mm16(pts[m][h], m, j, h)
                    last = j == NC8 - 1
                    for m in (0, 1, 2):
                        for h in (0, 1):
                            for q in (0, 1):
                                mm8(pts[m][h], m, j, h, q,
                                    stop=(last and q == 1))
                for m in (0, 1, 2):
                    for h in (0, 1):
                        emit_half(m, h, pts[m][h])

                # phase 2: m3..m7, halves staggered
                for m in range(3, MT):
                    lastm = m == MT - 1
                    pt = {h: ps.tile([P, NH], mybir.dt.float32, tag="ps", name=f"pt{m}_{h}")
                          for h in range(2)}
                    for h in (0, 1):
                        if lastm and h == 1:
                            break
                        for j in range(NC16):
                            mm16(pt[h], m, j, h, start=(j == 0))
                        for j in range(NC8):
                            for q in (0, 1):
                                mm8(pt[h], m, j, h, q,
                                    stop=(j == NC8 - 1 and q == 1))
                        emit_half(m, h, pt[h])
                # final half of m7: one quarter group + two eighth groups so
                # the tail chain after the very last matmul is short
                NQ = NH // 2  # 256
                NE = NH // 4  # 128
                ptq = ps.tile([P, NQ], mybir.dt.float32, tag="ps", name="ptq")
                for j in range(NC16):
                    nc.tensor.matmul(
                        ptq[:], xs16(MT - 1, j), w16t[j][:, NH:NH + NQ],
                        start=(j == 0), stop=False,
                    )
                for j in range(NC8):
                    nc.tensor.matmul(
                        ptq[:], xs8(MT - 1, j), w8t[j][:, :, NH:NH + NQ],
                        start=False, stop=(j == NC8 - 1), perf_mode=DR,
                    )
                emit_piece(MT - 1, NH, NH + NQ, ptq[:], nc.scalar)
                for e in range(2):
                    c0 = NH + NQ + e * NE
                    pte = ps.tile([P, NE], mybir.dt.float32, tag="ps", name=f"pte{e}")
                    for j in range(NC16):
                        nc.tensor.matmul(
                            pte[:], xs16(MT - 1, j), w16t[j][:, c0:c0 + NE],
                            start=(j == 0), stop=False,
                        )
                    for j in range(NC8):
                        nc.tensor.matmul(
                            pte[:], xs8(MT - 1, j), w8t[j][:, :, c0:c0 + NE],
                            start=False, stop=(j == NC8 - 1), perf_mode=DR,
                        )
                    emit_piece(MT - 1, c0, c0 + NE, pte[:],
                               nc.sync if e == 0 else nc.scalar)
    nc.compile()
    _cached_nc = nc
    return nc


def _construct_hamilton(A):
    r, i, j, k = A[:, 0], A[:, 1], A[:, 2], A[:, 3]
    return np.concatenate(
        [
            np.concatenate([r, -i, -j, -k], axis=2),
            np.concatenate([i, r, -k, j], axis=2),
            np.concatenate([j, k, r, -i], axis=2),
            np.concatenate([k, -j, i, r], axis=2),
        ],
        axis=1,
    )


def build_in_maps(x, A, factors_B):
    H = _construct_hamilton(np.asarray(A, dtype=np.float64))  # [r, k, s]
    Bf = np.asarray(factors_B, dtype=np.float64)  # [r, j, i]
    W = np.einsum("rks,rji->sikj", H, Bf).reshape(D, D)  # [k-row, col] f64

    Wr = W.reshape(8, P, D)
    w16 = np.ascontiguousarray(Wr[list(CH16)]).astype(np.float16)  # [NC16,P,D]
    w8 = np.empty((NC8, P, 2, D), dtype=F8)
    for ji, c in enumerate(CH8):
        Wc = Wr[c] / CX
        hi = Wc.astype(F8)
        lo = (Wc - hi.astype(np.float64)).astype(F8)
        w8[ji, :, 0] = hi
        w8[ji, :, 1] = lo

    x2 = np.asarray(x, dtype=np.float32).reshape(NCORES, MT, P, 8, P)
    in_maps = []
    for c in range(NCORES):
        xr = x2[c]  # [m, tok, chunk, k]
        x16 = np.ascontiguousarray(
            xr[:, :, list(CH16)].transpose(0, 3, 2, 1)
        ).astype(np.float16)  # [m, k, ci, tok]
        x8s = (xr[:, :, list(CH8)] * CX).astype(F8)  # [m, tok, ci, k]
        x8p = x8s.transpose(0, 3, 2, 1)[:, :, :, None, :]  # [m, k, ci, 1, tok]
        x8 = np.ascontiguousarray(
            np.broadcast_to(x8p, (MT, P, NC8, 2, P))
        )
        in_maps.append({
            "xH16": x16[:4],
            "xH8": x8[:4],
            "xB16": np.ascontiguousarray(x16[4:].transpose(1, 0, 2, 3)),
            "xB8": np.ascontiguousarray(x8[4:].transpose(1, 0, 2, 3, 4)),
            "wH16": w16,
            "wH8": w8,
        })
    return in_maps


def kernel(x, A, factors_B, bias):
    nc = build_module()
    in_maps = build_in_maps(x, A, factors_B)
    br = run_bass_kernel_spmd(nc, in_maps, core_ids=list(range(NCORES)))
    out = np.concatenate([r["out"] for r in br.results], axis=0)
    out = out.astype(np.float32) + np.asarray(bias, dtype=np.float32)[None, :]
    return out.reshape(B, T, D)


# revision 7
# speedup vs baseline: 1.1354x; 1.1354x over previous
"""TRN2 Bass kernel for nn_BalancedHamiltonLayer.

Math: out[n,k,j] = sum_{r,s,i} x[n,s,i] * factors_B[r,j,i] * H(A)[r,k,s] + bias
collapses to a single dense matmul  out = x2d @ W + bias  with
W[(s,i),(k,j)] = sum_r H[r,k,s] * B[r,j,i]  (1024x1024, folded on host in f64).

Sharding: data-parallel over the 8192 token rows across 8 NeuronCores
(1024 rows each); W replicated.  fp16 matmul (fp8 DoubleRow measured:
2x MAC rate but drops the PE clock 2.37->2.0GHz on 8 cores, a net loss),
fp32 PSUM accumulation, fp16 stores, bias added on host.

Schedule (per core, from NTFF traces):
- W (2MB) streams on the scalar HWDGE queue alone so it is never starved
  by x traffic; x (2MB) streams on sync, smallest pieces first.  First
  real matmul ~9.4us into the window.
- warmup matmuls on a zeroed tile (memset on the idle gpsimd engine)
  bridge body-start -> first data and ramp the PE HAM clock.
- phase 1 interleaves m0..m2 across k-chunks in W arrival order (6 MMs
  per 256KB W chunk = 1.4x slack vs the wire cadence); phase 2 runs
  m3..m7 k-contiguous per output half, stores draining on scalar (h0) /
  sync (h1).  The final half is a quarter + two eighth accumulation
  groups so the tail chain after the very last matmul is short.
"""

import numpy as np
import concourse.bacc as bacc
import concourse.mybir as mybir
import concourse.tile as tile
from concourse.bass_utils import run_bass_kernel_spmd

B, T, D = 4, 2048, 1024
RANK, FACTOR, SUB = 8, 64, 4
S = 4 * SUB  # 16
NCORES = 8
NTOK = B * T // NCORES  # 1024 token rows per core
P = 128
KT = D // P     # 8 contraction chunks
MT = NTOK // P  # 8 token tiles per core
NH = 512        # half of the 1024 output cols (one PSUM bank)

NWARM_BIG = 5
NWARM_SMALL = 3

_cached_nc = None


def build_module():
    global _cached_nc
    if _cached_nc is not None:
        return _cached_nc
    nc = bacc.Bacc("TRN2", target_bir_lowering=False, debug=False)
    xH = nc.dram_tensor("xH", [MT, P, KT, P], mybir.dt.float16, kind="ExternalInput").ap()
    wH = nc.dram_tensor("wH", [KT, P, D], mybir.dt.float16, kind="ExternalInput").ap()
    out = nc.dram_tensor("out", [NTOK, D], mybir.dt.float16, kind="ExternalOutput").ap()

    with tile.TileContext(nc) as tc:
        with (
            tc.tile_pool(name="wp", bufs=1) as wp,
            tc.tile_pool(name="xp", bufs=1) as xp,
            tc.tile_pool(name="op", bufs=1) as op,
            tc.tile_pool(name="ps", bufs=8, space="PSUM") as ps,
        ):
            g = xp.tile([P, NH], mybir.dt.float16, tag="warm", name="g")
            nc.gpsimd.memset(g[:], 0.0)

            wt = [wp.tile([P, D], mybir.dt.float16, tag=f"w{k}", name=f"w{k}")
                  for k in range(KT)]
            xt = [xp.tile([P, KT, P], mybir.dt.float16, tag=f"x{m}", name=f"x{m}")
                  for m in range(MT)]

            # W alone on the scalar queue (never starved by x); x on sync,
            # phase-1 k0 slices first so the first matmul can start early.
            nc.scalar.dma_start(wt[0][:, :NH], wH[0, :, :NH])
            nc.sync.dma_start(xt[0][:, 0:1], xH[0, :, 0:1])
            nc.scalar.dma_start(wt[0][:, NH:], wH[0, :, NH:])
            nc.sync.dma_start(xt[1][:, 0:1], xH[1, :, 0:1])
            nc.sync.dma_start(xt[2][:, 0:1], xH[2, :, 0:1])
            for k in range(1, KT):
                nc.scalar.dma_start(wt[k][:], wH[k])
            for m in range(3):
                nc.sync.dma_start(xt[m][:, 1:], xH[m, :, 1:])
            for m in range(3, MT):
                nc.sync.dma_start(xt[m][:], xH[m])

            ot = {}

            def emit_piece(m, c0, c1, pt_ap, eng):
                if m not in ot:
                    ot[m] = op.tile([P, D], mybir.dt.float16, tag=f"o{m}", name=f"o{m}")
                o = ot[m]
                nc.vector.tensor_copy(o[:, c0:c1], pt_ap)
                eng.dma_start(out[m * P:(m + 1) * P, c0:c1], o[:, c0:c1])

            def emit_half(m, h, pt):
                emit_piece(m, h * NH, (h + 1) * NH, pt[:],
                           nc.scalar if h == 0 else nc.sync)

            with nc.named_scope("mm"):
                pts = {
                    m: {h: ps.tile([P, NH], mybir.dt.float32, tag="ps", name=f"pt{m}_{h}")
                        for h in range(2)}
                    for m in range(3)
                }
                for i in range(NWARM_BIG):
                    nc.tensor.matmul(pts[0][0][:], g[:, :P], g[:], start=(i == 0), stop=False)
                for i in range(NWARM_SMALL):
                    nc.tensor.matmul(pts[0][0][:, :P], g[:, :P], g[:, :P], start=False, stop=False)

                def mm(m, k, h, pt=None, start=None, stop=None):
                    nc.tensor.matmul(
                        (pts[m][h] if pt is None else pt)[:],
                        xt[m][:, k, :],
                        wt[k][:, h * NH:(h + 1) * NH],
                        start=(k == 0 and not (m == 0 and h == 0)) if start is None else start,
                        stop=(k == KT - 1) if stop is None else stop,
                    )

                # phase 1: m0..m2 interleaved per chunk (6 MMs per W chunk)
                for k in range(KT):
                    for m in (0, 1, 2):
                        for h in (0, 1):
                            mm(m, k, h)
                for m in (0, 1, 2):
                    for h in (0, 1):
                        emit_half(m, h, pts[m][h])

                # phase 2: m3..m7, halves staggered
                for m in range(3, MT):
                    lastm = m == MT - 1
                    pt = {h: ps.tile([P, NH], mybir.dt.float32, tag="ps", name=f"pt{m}_{h}")
                          for h in range(2)}
                    for h in (0, 1):
                        if lastm and h == 1:
                            break
                        for k in range(KT):
                            mm(m, k, h, pt=pt[h], start=(k == 0), stop=(k == KT - 1))
                        emit_half(m, h, pt[h])
                # final half of m7: quarter + two eighth groups
                NQ, NE = NH // 2, NH // 4
                m = MT - 1
                ptq = ps.tile([P, NQ], mybir.dt.float32, tag="ps", name="ptq")
                for k in range(KT):
                    nc.tensor.matmul(
                        ptq[:], xt[m][:, k, :], wt[k][:, NH:NH + NQ],
                        start=(k == 0), stop=(k == KT - 1),
                    )
                emit_piece(m, NH, NH + NQ, ptq[:], nc.scalar)
                for e in range(2):
                    c0 = NH + NQ + e * NE
                    pte = ps.tile([P, NE], mybir.dt.float32, tag="ps", name=f"pte{e}")
                    for k in range(KT):
                        nc.tensor.matmul(
                            pte[:], xt[m][:, k, :], wt[k][:, c0:c0 + NE],
                            start=(k == 0), stop=(k == KT - 1),
                        )
                    emit_piece(m, c0, c0 + NE, pte[:],
                               nc.sync if e == 0 else nc.scalar)
    nc.compile()
    _cached_nc = nc
    return nc


def _construct_hamilton(A):
    r, i, j, k = A[:, 0], A[:, 1], A[:, 2], A[:, 3]
    return np.concatenate(
        [
            np.concatenate([r, -i, -j, -k], axis=2),
            np.concatenate([i, r, -k, j], axis=2),
            np.concatenate([j, k, r, -i], axis=2),
            np.concatenate([k, -j, i, r], axis=2),
        ],
        axis=1,
    )


def build_in_maps(x, A, factors_B):
    H = _construct_hamilton(np.asarray(A, dtype=np.float64))  # [r, k, s]
    Bf = np.asarray(factors_B, dtype=np.float64)  # [r, j, i]
    W = np.einsum("rks,rji->sikj", H, Bf).reshape(D, D).astype(np.float16)
    wH = np.ascontiguousarray(W.reshape(KT, P, D))

    x2 = np.asarray(x, dtype=np.float16).reshape(NCORES, NTOK, D)
    in_maps = []
    for c in range(NCORES):
        xs_ = np.ascontiguousarray(
            x2[c].reshape(MT, P, KT, P).transpose(0, 3, 2, 1)
        )
        in_maps.append({"xH": xs_, "wH": wH})
    return in_maps


def kernel(x, A, factors_B, bias):
    nc = build_module()
    in_maps = build_in_maps(x, A, factors_B)
    br = run_bass_kernel_spmd(nc, in_maps, core_ids=list(range(NCORES)))
    out = np.concatenate([r["out"] for r in br.results], axis=0)
    out = out.astype(np.float32) + np.asarray(bias, dtype=np.float32)[None, :]
    return out.reshape(B, T, D)


# revision 8
# speedup vs baseline: 1.1481x; 1.0112x over previous
"""TRN2 Bass kernel for nn_BalancedHamiltonLayer.

Math: out[n,k,j] = sum_{r,s,i} x[n,s,i] * factors_B[r,j,i] * H(A)[r,k,s] + bias
collapses to a single dense matmul  out = x2d @ W + bias  with
W[(s,i),(k,j)] = sum_r H[r,k,s] * B[r,j,i]  (1024x1024, folded on host in f64).

Sharding: data-parallel over the 8192 token rows across 8 NeuronCores
(1024 rows each); W replicated.  fp16 matmul (fp8 DoubleRow measured on
this HW: 2x MAC rate but drops the PE clock 2.37->2.0GHz when dense on
8 cores — a net loss), fp32 PSUM accumulation, fp16 stores, bias on host.

Schedule (per core, measured per-core DMA cap ~360GB/s shared across
queues, ~630-820ns per DMA issue, DGE start ~0.8us, completion->semaphore
~0.9us):
- W streams on the scalar HWDGE queue (w0 split in halves first), x on
  sync with tiny first-chunk slices so the first real matmul fires
  ~9.6us into the window (fixed preamble ends ~6.7us).  x tiles are 2-D
  [P, 2048B] so DMA descriptors stay large (3-D patterns degrade to
  256B descriptors at ~91GB/s).
- warmup matmuls on a zeroed tile (memset on vector) bridge body-start
  -> first data and ramp the PE HAM clock; a gap resets the clock to
  1.2GHz so warmup overshoots slightly.
- phase 1 k-interleaves m0,m1, joins m2 at k2 (like a prior tuned
  schedule), matching the ~0.9us/256KB wire cadence; phase 2 runs m3..m7
  k-contiguous per half, stores draining on scalar (h0) / sync (h1).
- the final half is a quarter + two eighth accumulation groups so the
  tail chain after the very last matmul is short.
"""

import numpy as np
import concourse.bacc as bacc
import concourse.mybir as mybir
import concourse.tile as tile
from concourse.bass_utils import run_bass_kernel_spmd

B, T, D = 4, 2048, 1024
RANK, FACTOR, SUB = 8, 64, 4
S = 4 * SUB  # 16
NCORES = 8
NTOK = B * T // NCORES  # 1024 token rows per core
P = 128
KT = D // P     # 8 contraction chunks
MT = NTOK // P  # 8 token tiles per core
NH = 512        # half of the 1024 output cols (one PSUM bank)

NWARM_BIG = 4
NWARM_SMALL = 4

_cached_nc = None


def build_module():
    global _cached_nc
    if _cached_nc is not None:
        return _cached_nc
    nc = bacc.Bacc("TRN2", target_bir_lowering=False, debug=False)
    xH = nc.dram_tensor("xH", [MT, P, KT * P], mybir.dt.float16, kind="ExternalInput").ap()
    wH = nc.dram_tensor("wH", [KT, P, D], mybir.dt.float16, kind="ExternalInput").ap()
    out = nc.dram_tensor("out", [NTOK, D], mybir.dt.float16, kind="ExternalOutput").ap()

    with tile.TileContext(nc) as tc:
        with (
            tc.tile_pool(name="wp", bufs=1) as wp,
            tc.tile_pool(name="xp", bufs=1) as xp,
            tc.tile_pool(name="op", bufs=1) as op,
            tc.tile_pool(name="ps", bufs=8, space="PSUM") as ps,
        ):
            g = xp.tile([P, NH], mybir.dt.float16, tag="warm", name="g")
            nc.vector.memset(g[:], 0.0)

            wt = [wp.tile([P, D], mybir.dt.float16, tag=f"w{k}", name=f"w{k}")
                  for k in range(KT)]
            xt = [xp.tile([P, KT * P], mybir.dt.float16, tag=f"x{m}", name=f"x{m}")
                  for m in range(MT)]

            # W on scalar (first chunk in halves); x on sync, k0 slices of
            # m0..m2 first, then rests / full tiles in consumption order.
            nc.scalar.dma_start(wt[0][:, :NH], wH[0, :, :NH])
            nc.sync.dma_start(xt[0][:, :P], xH[0, :, :P])
            nc.scalar.dma_start(wt[0][:, NH:], wH[0, :, NH:])
            nc.sync.dma_start(xt[0][:, P:2 * P], xH[0, :, P:2 * P])
            nc.sync.dma_start(xt[1][:, :P], xH[1, :, :P])
            nc.sync.dma_start(xt[2][:, :P], xH[2, :, :P])
            for k in range(1, KT):
                nc.scalar.dma_start(wt[k][:], wH[k])
            nc.sync.dma_start(xt[0][:, 2 * P:], xH[0, :, 2 * P:])
            nc.sync.dma_start(xt[1][:, P:], xH[1, :, P:])
            nc.sync.dma_start(xt[2][:, P:], xH[2, :, P:])
            for m in range(3, MT):
                nc.sync.dma_start(xt[m][:], xH[m])

            def xs(m, k):
                return xt[m][:, k * P:(k + 1) * P]

            ot = {}

            def emit_piece(m, c0, c1, pt_ap, eng):
                if m not in ot:
                    ot[m] = op.tile([P, D], mybir.dt.float16, tag=f"o{m}", name=f"o{m}")
                o = ot[m]
                nc.vector.tensor_copy(o[:, c0:c1], pt_ap)
                eng.dma_start(out[m * P:(m + 1) * P, c0:c1], o[:, c0:c1])

            def emit_half(m, h, pt):
                emit_piece(m, h * NH, (h + 1) * NH, pt[:],
                           nc.scalar if h == 0 else nc.sync)

            with nc.named_scope("mm"):
                pts = {
                    m: {h: ps.tile([P, NH], mybir.dt.float32, tag="ps", name=f"pt{m}_{h}")
                        for h in range(2)}
                    for m in range(3)
                }
                for i in range(NWARM_BIG):
                    nc.tensor.matmul(pts[0][0][:], g[:, :P], g[:], start=(i == 0), stop=False)
                for i in range(NWARM_SMALL):
                    nc.tensor.matmul(pts[0][0][:, :P], g[:, :P], g[:, :P], start=False, stop=False)

                def mm(m, k, h):
                    nc.tensor.matmul(
                        pts[m][h][:],
                        xs(m, k),
                        wt[k][:, h * NH:(h + 1) * NH],
                        start=(k == 0 and not (m == 0 and h == 0)),
                        stop=(k == KT - 1),
                    )

                # phase 1: m0,m1 lead k0..k1; m2 catches up; then 3-way
                for k in (0, 1):
                    for m in (0, 1):
                        for h in (0, 1):
                            mm(m, k, h)
                for k in (0, 1):
                    for h in (0, 1):
                        mm(2, k, h)
                for k in range(2, KT):
                    for m in (0, 1, 2):
                        for h in (0, 1):
                            mm(m, k, h)
                for m in (0, 1, 2):
                    for h in (0, 1):
                        emit_half(m, h, pts[m][h])

                # phase 2: m3..m7, halves staggered
                for m in range(3, MT):
                    lastm = m == MT - 1
                    pt = {h: ps.tile([P, NH], mybir.dt.float32, tag="ps", name=f"pt{m}_{h}")
                          for h in range(2)}
                    for h in (0, 1):
                        if lastm and h == 1:
                            break
                        for k in range(KT):
                            nc.tensor.matmul(
                                pt[h][:], xs(m, k), wt[k][:, h * NH:(h + 1) * NH],
                                start=(k == 0), stop=(k == KT - 1),
                            )
                        emit_half(m, h, pt[h])
                # final half of m7: quarter + two eighth groups
                NQ, NE = NH // 2, NH // 4
                m = MT - 1
                ptq = ps.tile([P, NQ], mybir.dt.float32, tag="ps", name="ptq")
                for k in range(KT):
                    nc.tensor.matmul(
                        ptq[:], xs(m, k), wt[k][:, NH:NH + NQ],
                        start=(k == 0), stop=(k == KT - 1),
                    )
                emit_piece(m, NH, NH + NQ, ptq[:], nc.scalar)
                for e in range(2):
                    c0 = NH + NQ + e * NE
                    pte = ps.tile([P, NE], mybir.dt.float32, tag="ps", name=f"pte{e}")
                    for k in range(KT):
                        nc.tensor.matmul(
                            pte[:], xs(m, k), wt[k][:, c0:c0 + NE],
                            start=(k == 0), stop=(k == KT - 1),
                        )
                    emit_piece(m, c0, c0 + NE, pte[:],
                               nc.sync if e == 0 else nc.scalar)
    nc.compile()
    _cached_nc = nc
    return nc


def _construct_hamilton(A):
    r, i, j, k = A[:, 0], A[:, 1], A[:, 2], A[:, 3]
    return np.concatenate(
        [
            np.concatenate([r, -i, -j, -k], axis=2),
            np.concatenate([i, r, -k, j], axis=2),
            np.concatenate([j, k, r, -i], axis=2),
            np.concatenate([k, -j, i, r], axis=2),
        ],
        axis=1,
    )


def build_in_maps(x, A, factors_B):
    H = _construct_hamilton(np.asarray(A, dtype=np.float64))  # [r, k, s]
    Bf = np.asarray(factors_B, dtype=np.float64)  # [r, j, i]
    W = np.einsum("rks,rji->sikj", H, Bf).reshape(D, D).astype(np.float16)
    wH = np.ascontiguousarray(W.reshape(KT, P, D))

    x2 = np.asarray(x, dtype=np.float16).reshape(NCORES, NTOK, D)
    in_maps = []
    for c in range(NCORES):
        xs_ = np.ascontiguousarray(
            x2[c].reshape(MT, P, KT, P).transpose(0, 3, 2, 1).reshape(MT, P, KT * P)
        )
        in_maps.append({"xH": xs_, "wH": wH})
    return in_maps


def kernel(x, A, factors_B, bias):
    nc = build_module()
    in_maps = build_in_maps(x, A, factors_B)
    br = run_bass_kernel_spmd(nc, in_maps, core_ids=list(range(NCORES)))
    out = np.concatenate([r["out"] for r in br.results], axis=0)
    out = out.astype(np.float32) + np.asarray(bias, dtype=np.float32)[None, :]
    return out.reshape(B, T, D)
